# revision 11
# baseline (speedup 1.0000x reference)
"""Causal dot-product attention on 8 Trainium2 NeuronCores.

Problem: q,k,v [16, 2048, 128] fp32, causal softmax(q k^T / sqrt(128)) v.
Sharding: heads (N=16) split across 8 cores, 2 heads per core; no cross-core
communication.

Per-core kernel design (two heads, one per pass, pipelined):
  - Everything downstream of the input DMA runs in bf16 (rel-err budget is
    2e-2; bf16 QK adds ~5e-3): warm-path Q/K/V loads are SWDGE DMAs that
    cast fp32->bf16 in flight, and the Q/K transposes to [F, T] run on the
    PE at bf16's 1 cycle/row (fp32 transposes are two-pass, ~4 cycles/row).
  - PSUM is one big [128, 6, 512] fp32 scores tile (6 banks) + a 2-bank
    accumulator tile. Score j-pairs cycle through the 3 row-pairs of the
    scores tile, giving the QK->exp->reuse chain a depth-3 ring (depth 2
    provably collapses into engine lockstep AND drops the PE's HAM clock
    from 2.4 to 1.2 GHz: the p-state gate needs a ~full activity window).
  - Transposes stage through the same ring: each chunk's 8 bf16 transposes
    write a bf16-bitcast view of one ring row-pair, so they inherit the
    ring's write-after-exp ordering and never stall the PE.
  - Consecutive off-diagonal j-pairs whose ring slots are address-adjacent
    get ONE merged [4, 512] exp: the scalar engine's ACTIVATE costs
    (N + 352 cycles), so halving the instruction count matters. Diagonal
    pairs keep their causal width trim (d2 is 256 wide) and stay single.
  - The causal band of diagonal tiles is zeroed post-exp by gpsimd
    affine_select (index-based, data-independent; lanes the trimmed QK left
    stale are never read by AV).
  - out[q, f+1] accumulates expT_ij.T @ [v_j | 1] over j into 2 PSUM banks;
    column 128 is the softmax denominator (ones column in v_aug), so no
    separate row-sum pass exists. Normalize = reciprocal + scalar-mul on
    the DVE, deferred off the block-boundary critical path.
  - AV matmuls are deferred a few units behind QK+exp so the in-order PE
    queue always holds ready work while exp runs; chunk loads are injected
    after each block's first unit.
  - Cold start: chunk-0 K/Q go over HWDGE in fp32 (SWDGE would serialize
    behind identity creation on gpsimd) and are fp32-transposed; the DVE
    copy into kT/qT rounds to bf16. PE clock-ramp warmup transposes run on
    a dummy tile right after the identity is ready.
"""

import numpy as np

import concourse.bass as bass
import concourse.mybir as mybir
import concourse.tile as tile
from concourse import bacc
from concourse.bass import ts
from concourse.bass_utils import run_bass_kernel_spmd
from concourse.masks import make_identity
from concourse.tile_rust import add_dep_helper

N, T, F = 16, 2048, 128
N_CORES = 8
H = N // N_CORES  # heads per core
P = 128
NT = T // P  # 16 k/q tiles per head
BLK = 4  # q-tiles per block (512 q columns)
NBLK = NT // BLK
SCALE = 1.0 / float(np.sqrt(F))
F32 = mybir.dt.float32
BF16 = mybir.dt.bfloat16

AV_DEPTH = 3  # deferred-AV depth in score units


def build(masked: bool):
    nc = bacc.Bacc("TRN2", target_bir_lowering=False, debug=False, num_devices=N_CORES)
    q = nc.dram_tensor("q", [H, T, F], F32, kind="ExternalInput")
    k = nc.dram_tensor("k", [H, T, F], F32, kind="ExternalInput")
    v = nc.dram_tensor("v", [H, T, F], F32, kind="ExternalInput")
    out = nc.dram_tensor("out", [H, T, F], F32, kind="ExternalOutput")

    with tile.TileContext(nc) as tc:
        _attention(tc, out, q, k, v, masked)
    nc.compile()
    return nc


def build_units(b: int, masked: bool):
    """Score units (j-pairs) for block b: (js, qk_lo per j, exp_lo).
    "inj" marks the chunk-load injection point."""
    if not masked:
        pairs = [([2 * p, 2 * p + 1], [0, 0], 0) for p in range(NT // 2)]
    else:
        d = 4 * b
        pairs = [([2 * p, 2 * p + 1], [0, 0], 0) for p in range(2 * b)]
        pairs.append(([d + 0, d + 1], [0, 128], 0))
        pairs.append(([d + 2, d + 3], [256, 384], 256))
    units = [("s",) + pairs[0], ("inj",)]
    units += [("s",) + p for p in pairs[1:]]
    return units


def _attention(tc, out, q, k, v, masked: bool):
    from contextlib import ExitStack

    nc = tc.nc
    ctx = ExitStack()
    consts = ctx.enter_context(tc.tile_pool(name="consts", bufs=1))
    natf_pool = ctx.enter_context(tc.tile_pool(name="natf", bufs=2))
    nat_pool = ctx.enter_context(tc.tile_pool(name="nat", bufs=4))
    big_pool = ctx.enter_context(tc.tile_pool(name="big", bufs=2))
    vpool = ctx.enter_context(tc.tile_pool(name="vpool", bufs=2))
    exp_pool = ctx.enter_context(tc.tile_pool(name="expp", bufs=6))
    osb_pool = ctx.enter_context(tc.tile_pool(name="osb", bufs=2))
    rec_pool = ctx.enter_context(tc.tile_pool(name="rec", bufs=4))
    ps_s = ctx.enter_context(tc.tile_pool(name="ps_s", bufs=1, space="PSUM"))
    ps_acc = ctx.enter_context(tc.tile_pool(name="ps_acc", bufs=1, space="PSUM"))

    q_ap, k_ap, v_ap, out_ap = q[:], k[:], v[:], out[:]
    CH = 4  # tiles per dma/transpose chunk (= one q-block's worth)

    # --- scores ring: one 6-bank PSUM tile, 3 row-pair slots ---
    ring = {"scores": None, "slot": 0}

    def new_scores():
        ring["scores"] = ps_s.tile([P, 6, 512], F32, tag="s", name="scores")
        ring["slot"] = 0

    def take_slot():
        a = ring["slot"]
        ring["slot"] = (a + 1) % 3
        return a

    def mk_state(n):
        st = {
            "n": n,
            "kr3": k_ap[n].rearrange("(j p) f -> p j f", p=P),
            "qr3": q_ap[n].rearrange("(j p) f -> p j f", p=P),
            "vr3": v_ap[n].rearrange("(j p) f -> p j f", p=P),
            "kT": big_pool.tile([P, T], BF16, tag="kT", name="kT"),
            "qT": big_pool.tile([P, T], BF16, tag="qT", name="qT"),
            "v_aug": vpool.tile([P, NT, P + 1], BF16, tag="vaug", name="v_aug"),
            "out_sb": osb_pool.tile([P, NT, P], F32, tag="osb", name="out_sb"),
        }
        nc.vector.memset(st["v_aug"][:, :, P : P + 1], 1.0)
        return st

    def dma_v_chunk(st, c):
        # SWDGE casts fp32 -> bf16 in flight
        nc.gpsimd.dma_start(
            out=st["v_aug"][:, c * CH : (c + 1) * CH, 0:P],
            in_=st["vr3"][:, c * CH : (c + 1) * CH, :],
        )

    def dma_chunk_bf(r3, c):
        nat = nat_pool.tile([P, CH, P], BF16, tag="nat")
        nc.gpsimd.dma_start(out=nat[:], in_=r3[:, c * CH : (c + 1) * CH, :])
        return nat

    def transpose_chunks_bf(kn, qn, st, c):
        """PE-transpose one k-chunk and one q-chunk (bf16, 1 cycle/row)
        through a bf16 view of one scores-ring row-pair."""
        a = take_slot()
        rb = ring["scores"][:, 2 * a : 2 * a + 2, :].bitcast(BF16)  # [P,2,1024]
        if kn is not None:
            for u in range(CH):
                nc.tensor.transpose(rb[:, 0, 128 * u : 128 * (u + 1)], kn[:, u, :], ident_b)
            nc.vector.tensor_copy(
                st["kT"][:, c * CH * P : (c + 1) * CH * P], rb[:, 0, 0:512]
            )
        if qn is not None:
            for u in range(CH):
                nc.tensor.transpose(rb[:, 1, 128 * u : 128 * (u + 1)], qn[:, u, :], ident_b)
            nc.vector.tensor_copy(
                st["qT"][:, c * CH * P : (c + 1) * CH * P], rb[:, 1, 0:512]
            )

    def load_chunks(st, c, kv=True):
        kn = None
        if kv:
            kn = dma_chunk_bf(st["kr3"], c)
            dma_v_chunk(st, c)
        qn = dma_chunk_bf(st["qr3"], c)
        transpose_chunks_bf(kn, qn, st, c)

    def normalize_and_store(st, acc_sb, b):
        rec4 = rec_pool.tile([P, BLK], F32, tag="rec")
        nc.vector.reciprocal(rec4[:], acc_sb[:, :, P : P + 1])
        for ii in range(BLK):
            i = BLK * b + ii
            nc.vector.tensor_scalar_mul(
                st["out_sb"][:, i, :], acc_sb[:, ii, 0:P], rec4[:, ii : ii + 1]
            )
        nc.sync.dma_start(
            out=out_ap[st["n"]].rearrange("(i p) f -> p i f", p=P)[
                :, BLK * b : BLK * (b + 1), :
            ],
            in_=st["out_sb"][:, BLK * b : BLK * (b + 1), :],
        )

    pending = []
    deferred = []

    def flush_one():
        av_fn, last_of_block, accs_, st_, b_ = deferred.pop(0)
        av_fn()
        if last_of_block:
            # evacuate accumulators; normalize is deferred further still
            acc_sb = rec_pool.tile([P, BLK, P + 1], F32, tag="accsb", name="acc_sb")
            nc.vector.tensor_copy(acc_sb[:], accs_[:, :, 0 : P + 1])
            pending.append((st_, acc_sb, b_))

    def flush_av():
        while deferred:
            flush_one()

    # ---- cold start ----
    st = mk_state(0)
    # chunk-0 K/Q over HWDGE (fp32): SWDGE would queue behind the identity
    # creation on gpsimd; separate rings so the two transfer in parallel
    kn0 = natf_pool.tile([P, CH, P], F32, tag="coldk")
    nc.sync.dma_start(out=kn0[:], in_=st["kr3"][:, 0:CH, :])
    qn0 = natf_pool.tile([P, CH, P], F32, tag="coldq")
    nc.scalar.dma_start(out=qn0[:], in_=st["qr3"][:, 0:CH, :])
    identity_f = consts.tile([P, P], F32)
    make_identity(nc, identity_f[:])
    dma_v_chunk(st, 0)
    identity = consts.tile([P, P], BF16)
    nc.vector.tensor_copy(identity[:], identity_f[:])  # fp32 -> bf16
    ident_b = identity[:]
    # touch Exp once at t=0 so the ~2.7us ACT table load overlaps the first
    # input DMA instead of delaying the first real exp
    warm = consts.tile([P, 1], F32)
    nc.scalar.activation(warm[:], identity_f[:, 0:1], mybir.ActivationFunctionType.Exp)
    new_scores()
    # warm the PE HAM clock gate while the first chunk DMAs land: dummy
    # transposes push the activity window over its busy threshold so the
    # real work runs at 2.4 GHz instead of 1.2
    wa = take_slot()
    wrow = ring["scores"][:, 2 * wa : 2 * wa + 2, :].bitcast(BF16)
    for u in range(8):
        nc.tensor.transpose(wrow[:, 0, 0:P], ident_b, ident_b)
    # chunk-0 transposes run in fp32 (two-pass) on the natural-layout fp32
    # tiles; the DVE copy rounds to bf16
    ca = take_slot()
    crow = ring["scores"][:, 2 * ca : 2 * ca + 2, :]
    for u in range(CH):
        nc.tensor.transpose(crow[:, 0, 128 * u : 128 * (u + 1)], kn0[:, u, :], identity_f[:])
    nc.vector.tensor_copy(st["kT"][:, 0 : CH * P], crow[:, 0, :])
    for u in range(CH):
        nc.tensor.transpose(crow[:, 1, 128 * u : 128 * (u + 1)], qn0[:, u, :], identity_f[:])
    nc.vector.tensor_copy(st["qT"][:, 0 : CH * P], crow[:, 1, :])

    st_next = None
    for n in range(H):
        if st is None:
            st, st_next = st_next, None
            new_scores()
        if not masked:
            for c in range(1, NBLK):
                kn = dma_chunk_bf(st["kr3"], c)
                dma_v_chunk(st, c)
                transpose_chunks_bf(kn, None, st, c)
        for b in range(NBLK):
            units = build_units(b, masked)
            sunits = [(ui, u) for ui, u in enumerate(units) if u[0] == "s"]
            # last (unit, j) hitting each accumulator, for stop flags
            last_map = {}
            for ui, (_, js, _, _) in sunits:
                for j in js:
                    for ii in range(BLK):
                        if not masked or j <= BLK * b + ii:
                            last_map[ii] = (ui, j)
            # Accumulators all share 2 PSUM banks at 256-fp32 stride.
            # start=True clears the whole bank's has_written bits, so only
            # the first j=0 matmul of each BANK starts (clearing the bank);
            # the neighbour accumulator's j=0 matmul is explicitly ordered
            # after it and overwrites (its hw bit was just cleared).
            accs = ps_acc.tile([P, BLK, 256], F32, tag="acc")  # 2 PSUM banks
            bank_first = {}

            def emit_qk(u, a):
                _, js, qk_lo, _ = u
                rows = ring["scores"][:, 2 * a : 2 * a + 2, :]
                for r, j in enumerate(js):
                    lo = qk_lo[r]
                    nc.tensor.matmul(
                        rows[:, r, lo:512],
                        lhsT=st["kT"][:, ts(j, P)],
                        rhs=st["qT"][:, 512 * b + lo : 512 * (b + 1)],
                        start=True,
                        stop=True,
                    )

            def emit_affine(u, expT, ro):
                if not masked:
                    return
                _, js, _, _ = u
                for r, j in enumerate(js):
                    ii = j - BLK * b
                    if 0 <= ii < BLK:
                        # zero the non-causal band of diagonal tiles post-exp
                        # on the otherwise-idle gpsimd
                        nc.gpsimd.affine_select(
                            out=expT[:, ro + r, ts(ii, P)],
                            in_=expT[:, ro + r, ts(ii, P)],
                            compare_op=mybir.AluOpType.is_ge,
                            fill=0.0,
                            base=0,
                            pattern=[[1, P]],
                            channel_multiplier=-1,
                        )

            def defer_av(u, ui, expT, ro):
                _, js, _, _ = u

                def av_fn(js=js, ui=ui, expT=expT, ro=ro, accs=accs, st=st, b=b,
                          bank_first=bank_first, last_map=last_map):
                    for r, j in enumerate(js):
                        for ii in range(BLK):
                            if masked and j > BLK * b + ii:
                                continue
                            bank = ii // 2
                            first = j == 0 and bank not in bank_first
                            m = nc.tensor.matmul(
                                accs[:, ii, 0 : P + 1],
                                lhsT=expT[:, ro + r, ts(ii, P)],
                                rhs=st["v_aug"][:, j, :],
                                start=first,
                                stop=last_map[ii] == (ui, j),
                                skip_group_check=True,
                            )
                            if first:
                                bank_first[bank] = m
                            elif j == 0:
                                # the bank-clearing start above must execute
                                # before this overwrite of the cleared bank
                                add_dep_helper(
                                    m.ins,
                                    bank_first[bank].ins,
                                    reason="acc bank clear precedes neighbour j0",
                                )

                deferred.append((av_fn, ui == sunits[-1][0], accs, st, b))

            ui = 0
            while ui < len(units):
                u = units[ui]
                if u[0] == "inj":
                    # previous blocks' deferred AV flushes here (ready PE
                    # meat); the last deferred unit's exp may still be in
                    # flight, so it flushes after the chunk loads
                    while len(deferred) > 1:
                        flush_one()
                    if b + 1 < NBLK:
                        load_chunks(st, b + 1, kv=masked)
                    elif n + 1 < H:
                        st_next = mk_state(n + 1)
                        load_chunks(st_next, 0)
                    flush_av()
                    while pending:
                        normalize_and_store(*pending.pop(0))
                    ui += 1
                    continue
                while len(deferred) >= AV_DEPTH:
                    flush_one()
                nxt = units[ui + 1] if ui + 1 < len(units) else None
                merge = (
                    nxt is not None
                    and nxt[0] == "s"
                    and ring["slot"] in (0, 1)
                    and u[3] == 0
                    and nxt[3] == 0
                )
                if merge:
                    a1 = take_slot()
                    a2 = take_slot()
                    emit_qk(u, a1)
                    emit_qk(nxt, a2)
                    expT = exp_pool.tile([P, 4, 512], BF16, tag="e")
                    nc.scalar.activation(
                        expT[:],
                        ring["scores"][:, 2 * a1 : 2 * a1 + 4, :],
                        mybir.ActivationFunctionType.Exp,
                        scale=SCALE,
                    )
                    emit_affine(u, expT, 0)
                    emit_affine(nxt, expT, 2)
                    defer_av(u, ui, expT, 0)
                    defer_av(nxt, ui + 1, expT, 2)
                    ui += 2
                else:
                    a = take_slot()
                    emit_qk(u, a)
                    lo = u[3]
                    expT = exp_pool.tile([P, 2, 512], BF16, tag="e")
                    nc.scalar.activation(
                        expT[:, :, lo:512],
                        ring["scores"][:, 2 * a : 2 * a + 2, lo:512],
                        mybir.ActivationFunctionType.Exp,
                        scale=SCALE,
                    )
                    emit_affine(u, expT, 0)
                    defer_av(u, ui, expT, 0)
                    ui += 1
        st = None
    flush_av()
    while pending:
        normalize_and_store(*pending.pop(0))

    ctx.close()


_CACHE = {}


def _get_nc(masked: bool):
    key = bool(masked)
    if key not in _CACHE:
        _CACHE[key] = build(key)
    return _CACHE[key]


def _run(q, k, v, masked, **kwargs):
    nc = _get_nc(masked)
    q = np.ascontiguousarray(np.asarray(q, dtype=np.float32))
    k = np.ascontiguousarray(np.asarray(k, dtype=np.float32))
    v = np.ascontiguousarray(np.asarray(v, dtype=np.float32))
    in_maps = [
        {
            "q": q[c * H : (c + 1) * H],
            "k": k[c * H : (c + 1) * H],
            "v": v[c * H : (c + 1) * H],
        }
        for c in range(N_CORES)
    ]
    res = run_bass_kernel_spmd(nc, in_maps, core_ids=list(range(N_CORES)), **kwargs)
    outs = np.concatenate([r["out"] for r in res.results], axis=0)
    return outs, res


def kernel(q, k, v, masked):
    m = int(np.asarray(masked))
    outs, _ = _run(q, k, v, m != 0)
    return outs


if __name__ == "__main__":
    rng = np.random.default_rng(0)
    qq = rng.standard_normal((N, T, F), dtype=np.float32)
    kk = rng.standard_normal((N, T, F), dtype=np.float32)
    vv = rng.standard_normal((N, T, F), dtype=np.float32)
    o = kernel(qq, kk, vv, 1)
    print("out", o.shape, o.dtype, float(np.abs(o).mean()))


# revision 12
# speedup vs baseline: 1.3429x; 1.3429x over previous
"""Causal dot-product attention on 8 Trainium2 NeuronCores.

Problem: q,k,v [16, 2048, 128] fp32, causal softmax(q k^T / sqrt(128)) v.
Sharding: heads (N=16) split across 8 cores, 2 heads per core; no cross-core
communication.

Per-core kernel design (two heads, one per pass, pipelined):
  - Warm-path Q/K/V loads are SWDGE DMAs casting fp32->bf16 in flight; Q/K
    are PE-transposed to [F, T] bf16 at 1 cycle/row (fp32 transposes are
    two-pass, ~4 cycles/row — they were 27% of the PE's work). The rel-err
    budget is 2e-2; bf16 QK costs ~3e-3.
  - Scores are computed transposed, scoresT[s, q] = kT_j.T @ qT, in pairs of
    k-tiles through 3 rotating 2-bank PSUM buffers; exp runs on the scalar
    engine (PSUM->SBUF, bf16 out, fused 1/sqrt(F) scale); the causal band of
    diagonal tiles is zeroed post-exp by gpsimd affine_select. QK matmuls
    are width-trimmed per-j on diagonal tiles (the exp of a trimmed lane
    reads stale-but-finite PSUM; those expT lanes are never consumed).
  - out[q, f+1] accumulates expT_ij.T @ [v_j | 1] over j into 2 PSUM banks
    (no start=True: a start clears the whole bank's has_written bits, so the
    banks are pre-zeroed and every matmul accumulates). Column 128 is the
    softmax denominator; normalize = per-partition reciprocal + scalar-mul,
    deferred off the block-boundary critical path.
  - Four-group software pipeline: each group's AV matmuls are emitted after
    the QK+exp of the next four groups, so the in-order PE queue always has
    ready QK work while exp runs. The 3-slot score ring keeps the PE
    streaming (a 2-deep ring provably collapses into engine lockstep and
    drops the PE HAM clock from 2.4 to 1.2 GHz).
  - Chunk loads/transposes for the next block (or next head) are injected
    mid-block so DMA + PE-transpose + DVE-copy overlap the matmul stream.
  - Cold start: chunk-0 K/Q DMAs are issued first (HWDGE, fp32, separate
    rings — SWDGE would queue behind identity creation on gpsimd), then the
    exp-table warm touch and PE clock-ramp dummies run while they land; the
    chunk-0 transposes are fp32 and the DVE copy rounds to bf16.
"""

import numpy as np

import concourse.bass as bass
import concourse.mybir as mybir
import concourse.tile as tile
from concourse import bacc
from concourse.bass import ts
from concourse.bass_utils import run_bass_kernel_spmd
from concourse.masks import make_identity
from concourse.tile_rust import add_dep_helper

N, T, F = 16, 2048, 128
N_CORES = 8
H = N // N_CORES  # heads per core
P = 128
NT = T // P  # 16 k/q tiles per head
BLK = 4  # q-tiles per block (512 q columns)
NBLK = NT // BLK
SCALE = 1.0 / float(np.sqrt(F))
F32 = mybir.dt.float32
BF16 = mybir.dt.bfloat16

# bf16 warm-path transposes (1 cycle/row). Flip to False to re-fatten the PE
# with fp32 two-pass transposes if the lighter PE under-runs the clock gate.
BF16_TP = True


def build(masked: bool):
    nc = bacc.Bacc("TRN2", target_bir_lowering=False, debug=False, num_devices=N_CORES)
    q = nc.dram_tensor("q", [H, T, F], F32, kind="ExternalInput")
    k = nc.dram_tensor("k", [H, T, F], F32, kind="ExternalInput")
    v = nc.dram_tensor("v", [H, T, F], F32, kind="ExternalInput")
    out = nc.dram_tensor("out", [H, T, F], F32, kind="ExternalOutput")

    with tile.TileContext(nc) as tc:
        _attention(tc, out, q, k, v, masked)
    nc.compile()
    return nc


def _attention(tc, out, q, k, v, masked: bool):
    from contextlib import ExitStack

    nc = tc.nc
    ctx = ExitStack()
    consts = ctx.enter_context(tc.tile_pool(name="consts", bufs=1))
    nat_pool = ctx.enter_context(tc.tile_pool(name="nat", bufs=4))
    big_pool = ctx.enter_context(tc.tile_pool(name="big", bufs=2))
    vpool = ctx.enter_context(tc.tile_pool(name="vpool", bufs=2))
    exp_pool = ctx.enter_context(tc.tile_pool(name="expp", bufs=7))
    osb_pool = ctx.enter_context(tc.tile_pool(name="osb", bufs=2))
    rec_pool = ctx.enter_context(tc.tile_pool(name="rec", bufs=4))
    ps_s = ctx.enter_context(tc.tile_pool(name="ps_s", bufs=3, space="PSUM"))
    ps_acc = ctx.enter_context(tc.tile_pool(name="ps_acc", bufs=1, space="PSUM"))

    q_ap, k_ap, v_ap, out_ap = q[:], k[:], v[:], out[:]
    CH = 4  # tiles per dma/transpose chunk (= one q-block's worth)

    def mk_state(n):
        st = {
            "n": n,
            "kr3": k_ap[n].rearrange("(j p) f -> p j f", p=P),
            "qr3": q_ap[n].rearrange("(j p) f -> p j f", p=P),
            "vr3": v_ap[n].rearrange("(j p) f -> p j f", p=P),
            "kT": big_pool.tile([P, T], BF16, tag="kT", name="kT"),
            "qT": big_pool.tile([P, T], BF16, tag="qT", name="qT"),
            "v_aug": vpool.tile([P, NT, P + 1], BF16, tag="vaug", name="v_aug"),
            "out_sb": osb_pool.tile([P, NT, P], F32, tag="osb", name="out_sb"),
        }
        nc.vector.memset(st["v_aug"][:, :, P : P + 1], 1.0)
        return st

    def dma_v_chunk(st, c):
        # SWDGE casts fp32 -> bf16 in flight
        nc.gpsimd.dma_start(
            out=st["v_aug"][:, c * CH : (c + 1) * CH, 0:P],
            in_=st["vr3"][:, c * CH : (c + 1) * CH, :],
        )

    def load_transpose_chunk(r3, dst, c):
        """SWDGE-cast 4 natural [128,128] tiles to bf16 and PE-transpose
        them into dst at 1 cycle/row."""
        if BF16_TP:
            nat = nat_pool.tile([P, CH, P], BF16, tag="nat")
            nc.gpsimd.dma_start(out=nat[:], in_=r3[:, c * CH : (c + 1) * CH, :])
            tp = ps_s.tile([P, CH, P], BF16, tag="s", name="tp")
            ident = ident_b
        else:
            nat = nat_pool.tile([P, CH, P], F32, tag="nat")
            nc.sync.dma_start(out=nat[:], in_=r3[:, c * CH : (c + 1) * CH, :])
            tp = ps_s.tile([P, CH, P], F32, tag="s", name="tp")
            ident = identity_f[:]
        for u in range(CH):
            nc.tensor.transpose(tp[:, u, :], nat[:, u, :], ident)
        nc.vector.tensor_copy(dst[:, c * CH * P : (c + 1) * CH * P], tp[:])

    def load_chunks(st, c, kv=True):
        if kv:
            load_transpose_chunk(st["kr3"], st["kT"], c)
            dma_v_chunk(st, c)
        load_transpose_chunk(st["qr3"], st["qT"], c)

    def normalize_and_store(st, acc_sb, b):
        rec4 = rec_pool.tile([P, BLK], F32, tag="rec")
        nc.vector.reciprocal(rec4[:], acc_sb[:, :, P : P + 1])
        for ii in range(BLK):
            i = BLK * b + ii
            nc.vector.tensor_scalar_mul(
                st["out_sb"][:, i, :], acc_sb[:, ii, 0:P], rec4[:, ii : ii + 1]
            )
        nc.sync.dma_start(
            out=out_ap[st["n"]].rearrange("(i p) f -> p i f", p=P)[
                :, BLK * b : BLK * (b + 1), :
            ],
            in_=st["out_sb"][:, BLK * b : BLK * (b + 1), :],
        )

    # ---- cold start: chunk-0 DMAs in flight before any warmup ----
    st = mk_state(0)
    kn0 = nat_pool.tile([P, CH, P], F32, tag="coldn")
    nc.sync.dma_start(out=kn0[:], in_=st["kr3"][:, 0:CH, :])
    qn0 = nat_pool.tile([P, CH, P], F32, tag="coldn")
    nc.scalar.dma_start(out=qn0[:], in_=st["qr3"][:, 0:CH, :])
    identity_f = consts.tile([P, P], F32)
    make_identity(nc, identity_f[:])
    dma_v_chunk(st, 0)
    identity = consts.tile([P, P], BF16)
    nc.vector.tensor_copy(identity[:], identity_f[:])  # fp32 -> bf16
    ident_b = identity[:]
    # touch Exp once at t=0 so the ~2.7us ACT table load overlaps the first
    # input DMA instead of delaying the first real exp
    warm = consts.tile([P, 1], F32)
    nc.scalar.activation(warm[:], identity_f[:, 0:1], mybir.ActivationFunctionType.Exp)
    # warm the PE HAM clock gate during the initial input-DMA wait: dummy
    # transposes push the activity window over its busy threshold so the
    # first real transposes/matmuls run at 2.4 GHz instead of 1.2
    wtp = ps_s.tile([P, P], F32, tag="s", name="wtp")
    for _ in range(6):
        nc.tensor.transpose(wtp[:], identity_f[:], identity_f[:])
    # chunk-0 transposes in fp32; the DVE copy rounds to bf16
    for (n0, dst) in ((kn0, st["kT"]), (qn0, st["qT"])):
        tp0 = ps_s.tile([P, CH, P], F32, tag="s", name="tp0")
        for u in range(CH):
            nc.tensor.transpose(tp0[:, u, :], n0[:, u, :], identity_f[:])
        nc.vector.tensor_copy(dst[:, 0 : CH * P], tp0[:])

    # ---- main loop: heads x 512-wide q blocks ----
    # j-tiles are processed in pairs through 3 rotating 2-bank PSUM score
    # buffers: QK of pair g+2, exp of pair g+1, and AV of pair g all run
    # concurrently.  Chunk loads for the next block (or next head) and the
    # previous block's normalize run mid-block, off the boundary handoff.
    pending = []
    st_next = None
    # four-group software pipeline: each group's AV matmuls are emitted after
    # the QK+exp of the next FOUR groups, so the in-order PE queue always has
    # ready QK work (including the next block's) while exp runs
    deferred = []
    AV_DEPTH = 4

    def flush_one():
        av_fn, last_of_block, accs_, st_, b_ = deferred.pop(0)
        av_fn()
        if last_of_block:
            # evacuate accumulators; normalize is deferred further still
            acc_sb = rec_pool.tile([P, BLK, P + 1], F32, tag="accsb", name="acc_sb")
            nc.vector.tensor_copy(acc_sb[:], accs_[:, :, 0 : P + 1])
            pending.append((st_, acc_sb, b_))

    def flush_av():
        while deferred:
            flush_one()

    for n in range(H):
        if st is None:
            st, st_next = st_next, None
        if not masked:
            for c in range(1, NBLK):
                load_transpose_chunk(st["kr3"], st["kT"], c)
                dma_v_chunk(st, c)
        for b in range(NBLK):
            n_j = 4 * (b + 1) if masked else NT
            # Accumulators all share 2 PSUM banks at 256-fp32 stride.
            # start=True clears the whole bank's has_written bits, so only
            # the first j=0 matmul of each BANK starts (clearing the bank);
            # the neighbour accumulator's j=0 matmul is explicitly ordered
            # after it and overwrites (its hw bit was just cleared).
            accs = ps_acc.tile([P, BLK, 256], F32, tag="acc")  # 2 PSUM banks
            bank_first = {}
            inject_at = max(2, (n_j // 2) & ~1)
            for g0 in range(0, n_j, 2):
                if g0 == inject_at:
                    # mid-block: previous block's normalize + next block's
                    # (or next head's) chunk loads run here, clear of the
                    # boundary handoff
                    while pending:
                        normalize_and_store(*pending.pop(0))
                    if b + 1 < NBLK:
                        load_chunks(st, b + 1, kv=masked)
                    elif n + 1 < H:
                        st_next = mk_state(n + 1)
                        load_chunks(st_next, 0)
                gsz = min(2, n_j - g0)
                # diagonal tiles only need the causal span of columns;
                # exp covers the pair's rectangular hull (stale lanes are
                # finite and never read by AV)
                pair_lo = 0
                if masked and g0 - 4 * b >= 0:
                    pair_lo = P * (g0 - 4 * b)
                scores = ps_s.tile([P, 2, 512], F32, tag="s")
                for r in range(gsz):
                    j = g0 + r
                    lo = pair_lo
                    if masked and j - 4 * b >= 0:
                        lo = P * (j - 4 * b)
                    nc.tensor.matmul(
                        scores[:, r, lo:512],
                        lhsT=st["kT"][:, ts(j, P)],
                        rhs=st["qT"][:, 512 * b + lo : 512 * (b + 1)],
                        start=True,
                        stop=True,
                    )
                expT = exp_pool.tile([P, 2, 512], BF16, tag="expT")
                nc.scalar.activation(
                    expT[:, 0:gsz, pair_lo:512],
                    scores[:, 0:gsz, pair_lo:512],
                    mybir.ActivationFunctionType.Exp,
                    scale=SCALE,
                )
                if masked:
                    # zero the upper-triangular (non-causal) band of any
                    # diagonal tile, post-exp, on the otherwise-idle gpsimd
                    for r in range(gsz):
                        ii = g0 + r - 4 * b
                        if 0 <= ii < BLK:
                            nc.gpsimd.affine_select(
                                out=expT[:, r, ts(ii, P)],
                                in_=expT[:, r, ts(ii, P)],
                                compare_op=mybir.AluOpType.is_ge,
                                fill=0.0,
                                base=0,
                                pattern=[[1, P]],
                                channel_multiplier=-1,
                            )
                while len(deferred) >= AV_DEPTH:
                    flush_one()

                def av_fn(expT=expT, g0=g0, gsz=gsz, accs=accs, st=st, b=b,
                          bank_first=bank_first):
                    for r in range(gsz):
                        j = g0 + r
                        for ii in range(BLK):
                            i = BLK * b + ii
                            if masked and j > i:
                                continue
                            bank = ii // 2
                            first = j == 0 and bank not in bank_first
                            m = nc.tensor.matmul(
                                accs[:, ii, 0 : P + 1],
                                lhsT=expT[:, r, ts(ii, P)],
                                rhs=st["v_aug"][:, j, :],
                                start=first,
                                stop=(j == (i if masked else NT - 1)),
                                skip_group_check=True,
                            )
                            if first:
                                bank_first[bank] = m
                            elif j == 0:
                                # the bank-clearing start above must execute
                                # before this overwrite of the cleared bank
                                add_dep_helper(
                                    m.ins,
                                    bank_first[bank].ins,
                                    reason="acc bank clear precedes neighbour j0",
                                )

                deferred.append((av_fn, g0 + 2 >= n_j, accs, st, b))
        st = None
    flush_av()
    while pending:
        normalize_and_store(*pending.pop(0))

    ctx.close()


_CACHE = {}


def _get_nc(masked: bool):
    key = bool(masked)
    if key not in _CACHE:
        _CACHE[key] = build(key)
    return _CACHE[key]


def _run(q, k, v, masked, **kwargs):
    nc = _get_nc(masked)
    q = np.ascontiguousarray(np.asarray(q, dtype=np.float32))
    k = np.ascontiguousarray(np.asarray(k, dtype=np.float32))
    v = np.ascontiguousarray(np.asarray(v, dtype=np.float32))
    in_maps = [
        {
            "q": q[c * H : (c + 1) * H],
            "k": k[c * H : (c + 1) * H],
            "v": v[c * H : (c + 1) * H],
        }
        for c in range(N_CORES)
    ]
    res = run_bass_kernel_spmd(nc, in_maps, core_ids=list(range(N_CORES)), **kwargs)
    outs = np.concatenate([r["out"] for r in res.results], axis=0)
    return outs, res


def kernel(q, k, v, masked):
    m = int(np.asarray(masked))
    outs, _ = _run(q, k, v, m != 0)
    return outs


if __name__ == "__main__":
    rng = np.random.default_rng(0)
    qq = rng.standard_normal((N, T, F), dtype=np.float32)
    kk = rng.standard_normal((N, T, F), dtype=np.float32)
    vv = rng.standard_normal((N, T, F), dtype=np.float32)
    o = kernel(qq, kk, vv, 1)
    print("out", o.shape, o.dtype, float(np.abs(o).mean()))


# revision 13
# speedup vs baseline: 1.5904x; 1.1843x over previous
"""Causal dot-product attention on 8 Trainium2 NeuronCores.

Problem: q,k,v [16, 2048, 128] fp32, causal softmax(q k^T / sqrt(128)) v.
Sharding: heads (N=16) split across 8 cores, 2 heads per core; no cross-core
communication.

Per-core kernel design (two heads, one per pass, pipelined):
  - Q and K are transposed to [F, T] float32r layout via chunked PE transposes
    (matmul contraction must sit on the partition dim; float32r streams at
    1 cycle/row vs fp32's 4). V is cast to bf16 with an all-ones column
    appended, so the attention matmul itself produces the softmax row-sums.
  - Scores are computed transposed, scoresT[s, q] = kT_j.T @ qT, in pairs of
    k-tiles through 3 rotating 2-bank PSUM buffers; exp runs on the scalar
    engine (PSUM->SBUF, bf16 out, fused 1/sqrt(F) scale); the causal band of
    diagonal tiles is zeroed post-exp by gpsimd affine_select.
  - out[q, f+1] accumulates expT_ij.T @ [v_j | 1] over j into 2 PSUM banks
    (no start=True: a start clears the whole bank's has_written bits, so the
    banks are pre-zeroed and every matmul accumulates). Column 128 is the
    softmax denominator; normalize = per-partition reciprocal + scalar-mul,
    deferred off the block-boundary critical path.
  - Chunk loads/transposes for the next block (or next head) are injected
    mid-block so DMA + PE-transpose + DVE-copy overlap the matmul stream.
"""

import numpy as np

import concourse.bass as bass
import concourse.mybir as mybir
import concourse.tile as tile
from concourse import bacc
from concourse.bass import ts
from concourse.bass_utils import run_bass_kernel_spmd
from concourse.masks import make_identity
from concourse.tile_rust import add_dep_helper

N, T, F = 16, 2048, 128
N_CORES = 8
H = N // N_CORES  # heads per core
P = 128
NT = T // P  # 16 k/q tiles per head
BLK = 4  # q-tiles per block (512 q columns)
NBLK = NT // BLK
SCALE = 1.0 / float(np.sqrt(F))
F32 = mybir.dt.float32
F32R = mybir.dt.float32r  # TF32-like PE mode: 1 cycle/row at N>=256 (fp32 is 4)
BF16 = mybir.dt.bfloat16


def build(masked: bool):
    nc = bacc.Bacc("TRN2", target_bir_lowering=False, debug=False, num_devices=N_CORES)
    q = nc.dram_tensor("q", [H, T, F], F32, kind="ExternalInput")
    k = nc.dram_tensor("k", [H, T, F], F32, kind="ExternalInput")
    v = nc.dram_tensor("v", [H, T, F], F32, kind="ExternalInput")
    out = nc.dram_tensor("out", [H, T, F], F32, kind="ExternalOutput")

    with tile.TileContext(nc) as tc:
        _attention(tc, out, q, k, v, masked)
    nc.compile()
    return nc


def _attention(tc, out, q, k, v, masked: bool):
    from contextlib import ExitStack

    nc = tc.nc
    ctx = ExitStack()
    consts = ctx.enter_context(tc.tile_pool(name="consts", bufs=1))
    nat_pool = ctx.enter_context(tc.tile_pool(name="nat", bufs=4))
    big_pool = ctx.enter_context(tc.tile_pool(name="big", bufs=2))
    vpool = ctx.enter_context(tc.tile_pool(name="vpool", bufs=2))
    exp_pool = ctx.enter_context(tc.tile_pool(name="expp", bufs=7))
    osb_pool = ctx.enter_context(tc.tile_pool(name="osb", bufs=2))
    rec_pool = ctx.enter_context(tc.tile_pool(name="rec", bufs=4))
    ps_s = ctx.enter_context(tc.tile_pool(name="ps_s", bufs=3, space="PSUM"))
    ps_acc = ctx.enter_context(tc.tile_pool(name="ps_acc", bufs=1, space="PSUM"))

    identity = consts.tile([P, P], F32)
    make_identity(nc, identity[:])
    # touch Exp once at t=0 so the ~2.7us ACT table load overlaps the first
    # input DMA instead of delaying the first real exp
    warm = consts.tile([P, 1], F32)
    nc.scalar.activation(warm[:], identity[:, 0:1], mybir.ActivationFunctionType.Exp)
    # warm the PE HAM clock gate during the initial input-DMA wait: ~2us of
    # dummy transposes push the activity window over its busy threshold so
    # the first real transposes/matmuls run at 2.4 GHz instead of 1.2
    wtp = ps_s.tile([P, P], F32, tag="s", name="wtp")
    for _ in range(6):
        nc.tensor.transpose(wtp[:], identity[:], identity[:])

    q_ap, k_ap, v_ap, out_ap = q[:], k[:], v[:], out[:]
    CH = 4  # tiles per dma/transpose chunk (= one q-block's worth)

    def load_transpose_chunk(r3, dst, c, eng=None):
        """DMA 4 natural [128,128] tiles and PE-transpose them into dst.

        eng picks the issuing HWDGE ring — HWDGE DMAs are FIFO per issuing
        engine, so the cold-start K and Q chunks go on different rings
        (sync vs scalar) to transfer in parallel.
        """
        nat = nat_pool.tile([P, CH, P], F32, tag="nat")
        (eng or nc.sync).dma_start(
            out=nat[:], in_=r3[:, c * CH : (c + 1) * CH, :]
        )
        tp = ps_s.tile([P, CH, P], F32, tag="s")
        for u in range(CH):
            nc.tensor.transpose(tp[:, u, :], nat[:, u, :], identity[:])
        nc.vector.tensor_copy(dst[:, c * CH * P : (c + 1) * CH * P], tp[:])

    def mk_state(n):
        st = {
            "n": n,
            "kr3": k_ap[n].rearrange("(j p) f -> p j f", p=P),
            "qr3": q_ap[n].rearrange("(j p) f -> p j f", p=P),
            "vr3": v_ap[n].rearrange("(j p) f -> p j f", p=P),
            "kT": big_pool.tile([P, T], F32R, tag="kT", name="kT"),
            "qT": big_pool.tile([P, T], F32R, tag="qT", name="qT"),
            "v_aug": vpool.tile([P, NT, P + 1], BF16, tag="vaug", name="v_aug"),
            "out_sb": osb_pool.tile([P, NT, P], F32, tag="osb", name="out_sb"),
        }
        nc.vector.memset(st["v_aug"][:, :, P : P + 1], 1.0)
        return st

    def load_chunks(st, c, kv=True, cold=False):
        if kv:
            load_transpose_chunk(st["kr3"], st["kT"], c)
            # SWDGE casts fp32 -> bf16 in flight
            nc.gpsimd.dma_start(
                out=st["v_aug"][:, c * CH : (c + 1) * CH, 0:P],
                in_=st["vr3"][:, c * CH : (c + 1) * CH, :],
            )
        load_transpose_chunk(
            st["qr3"], st["qT"], c, eng=nc.scalar if cold else None
        )

    def normalize_and_store(st, acc_sb, b):
        rec4 = rec_pool.tile([P, BLK], F32, tag="rec")
        nc.vector.reciprocal(rec4[:], acc_sb[:, :, P : P + 1])
        for ii in range(BLK):
            i = BLK * b + ii
            nc.vector.tensor_scalar_mul(
                st["out_sb"][:, i, :], acc_sb[:, ii, 0:P], rec4[:, ii : ii + 1]
            )
        nc.sync.dma_start(
            out=out_ap[st["n"]].rearrange("(i p) f -> p i f", p=P)[
                :, BLK * b : BLK * (b + 1), :
            ],
            in_=st["out_sb"][:, BLK * b : BLK * (b + 1), :],
        )

    # ---- main loop: heads x 512-wide q blocks ----
    # j-tiles are processed in pairs through 3 rotating 2-bank PSUM score
    # buffers: QK of pair g+2, exp of pair g+1, and AV of pair g all run
    # concurrently.  Chunk loads for the next block (or next head) and the
    # previous block's normalize run mid-block, off the boundary handoff.
    pending = []
    st = None
    st_next = None
    # four-group software pipeline: each group's AV matmuls are emitted after
    # the QK+exp of the next FOUR groups, so the in-order PE queue always has
    # ready QK work (including the next block's) while exp runs
    deferred = []
    AV_DEPTH = 4

    def flush_one():
        nonlocal pending
        av_fn, last_of_block, accs_, st_, b_ = deferred.pop(0)
        av_fn()
        if last_of_block:
            # evacuate accumulators; normalize is deferred further still
            acc_sb = rec_pool.tile([P, BLK, P + 1], F32, tag="accsb", name="acc_sb")
            nc.vector.tensor_copy(acc_sb[:], accs_[:, :, 0 : P + 1])
            pending.append((st_, acc_sb, b_))

    def flush_av():
        while deferred:
            flush_one()

    for n in range(H):
        st, st_next = st_next, None
        if st is None:
            st = mk_state(n)
            load_chunks(st, 0, cold=True)
        if not masked:
            for c in range(1, NBLK):
                load_transpose_chunk(st["kr3"], st["kT"], c)
                nc.gpsimd.dma_start(
                    out=st["v_aug"][:, c * CH : (c + 1) * CH, 0:P],
                    in_=st["vr3"][:, c * CH : (c + 1) * CH, :],
                )
        for b in range(NBLK):
            n_j = 4 * (b + 1) if masked else NT
            # Accumulators all share 2 PSUM banks at 256-fp32 stride.
            # start=True clears the whole bank's has_written bits, so only
            # the first j=0 matmul of each BANK starts (clearing the bank);
            # the neighbour accumulator's j=0 matmul is explicitly ordered
            # after it and overwrites (its hw bit was just cleared).
            accs = ps_acc.tile([P, BLK, 256], F32, tag="acc")  # 2 PSUM banks
            bank_first = {}
            inject_at = max(2, (n_j // 2) & ~1)
            for g0 in range(0, n_j, 2):
                if g0 == inject_at:
                    # mid-block: previous block's normalize + next block's
                    # (or next head's) chunk loads run here, clear of the
                    # boundary handoff
                    while pending:
                        normalize_and_store(*pending.pop(0))
                    if b + 1 < NBLK:
                        load_chunks(st, b + 1, kv=masked)
                    elif n + 1 < H:
                        st_next = mk_state(n + 1)
                        load_chunks(st_next, 0)
                gsz = min(2, n_j - g0)
                # diagonal pairs only need the causal span of columns
                col_lo = 0
                if masked and g0 - 4 * b >= 0:
                    col_lo = P * (g0 - 4 * b)
                scores = ps_s.tile([P, 2, 512], F32, tag="s")
                for r in range(gsz):
                    j = g0 + r
                    nc.tensor.matmul(
                        scores[:, r, col_lo:512],
                        lhsT=st["kT"][:, ts(j, P)],
                        rhs=st["qT"][:, 512 * b + col_lo : 512 * (b + 1)],
                        start=True,
                        stop=True,
                    )
                expT = exp_pool.tile([P, 2, 512], BF16, tag="expT")
                nc.scalar.activation(
                    expT[:, 0:gsz, col_lo:512],
                    scores[:, 0:gsz, col_lo:512],
                    mybir.ActivationFunctionType.Exp,
                    scale=SCALE,
                )
                if masked:
                    # zero the upper-triangular (non-causal) band of any
                    # diagonal tile, post-exp, on the otherwise-idle gpsimd
                    for r in range(gsz):
                        ii = g0 + r - 4 * b
                        if 0 <= ii < BLK:
                            nc.gpsimd.affine_select(
                                out=expT[:, r, ts(ii, P)],
                                in_=expT[:, r, ts(ii, P)],
                                compare_op=mybir.AluOpType.is_ge,
                                fill=0.0,
                                base=0,
                                pattern=[[1, P]],
                                channel_multiplier=-1,
                            )
                while len(deferred) >= AV_DEPTH:
                    flush_one()

                def av_fn(expT=expT, g0=g0, gsz=gsz, accs=accs, st=st, b=b,
                          bank_first=bank_first):
                    for r in range(gsz):
                        j = g0 + r
                        for ii in range(BLK):
                            i = BLK * b + ii
                            if masked and j > i:
                                continue
                            bank = ii // 2
                            first = j == 0 and bank not in bank_first
                            m = nc.tensor.matmul(
                                accs[:, ii, 0 : P + 1],
                                lhsT=expT[:, r, ts(ii, P)],
                                rhs=st["v_aug"][:, j, :],
                                start=first,
                                stop=(j == (i if masked else NT - 1)),
                                skip_group_check=True,
                            )
                            if first:
                                bank_first[bank] = m
                            elif j == 0:
                                # the bank-clearing start above must execute
                                # before this overwrite of the cleared bank
                                add_dep_helper(
                                    m.ins,
                                    bank_first[bank].ins,
                                    reason="acc bank clear precedes neighbour j0",
                                )

                deferred.append((av_fn, g0 + 2 >= n_j, accs, st, b))
    flush_av()
    while pending:
        normalize_and_store(*pending.pop(0))

    ctx.close()


_CACHE = {}


def _get_nc(masked: bool):
    key = bool(masked)
    if key not in _CACHE:
        _CACHE[key] = build(key)
    return _CACHE[key]


def _run(q, k, v, masked, **kwargs):
    nc = _get_nc(masked)
    q = np.ascontiguousarray(np.asarray(q, dtype=np.float32))
    k = np.ascontiguousarray(np.asarray(k, dtype=np.float32))
    v = np.ascontiguousarray(np.asarray(v, dtype=np.float32))
    in_maps = [
        {
            "q": q[c * H : (c + 1) * H],
            "k": k[c * H : (c + 1) * H],
            "v": v[c * H : (c + 1) * H],
        }
        for c in range(N_CORES)
    ]
    res = run_bass_kernel_spmd(nc, in_maps, core_ids=list(range(N_CORES)), **kwargs)
    outs = np.concatenate([r["out"] for r in res.results], axis=0)
    return outs, res


def kernel(q, k, v, masked):
    m = int(np.asarray(masked))
    outs, _ = _run(q, k, v, m != 0)
    return outs


if __name__ == "__main__":
    rng = np.random.default_rng(0)
    qq = rng.standard_normal((N, T, F), dtype=np.float32)
    kk = rng.standard_normal((N, T, F), dtype=np.float32)
    vv = rng.standard_normal((N, T, F), dtype=np.float32)
    o = kernel(qq, kk, vv, 1)
    print("out", o.shape, o.dtype, float(np.abs(o).mean()))

